# revision 4
# baseline (speedup 1.0000x reference)
"""MultiHeadGAT layer on 8 Trainium2 NeuronCores — v2 (dst-row layout).

Strategy (graph/data parallel, dst-sharded, per your sharding hint):
  - Nodes partitioned into 8 contiguous ranges (6250/core). Within a core,
    nodes are PERMUTED by descending degree and packed into 49 tiles of 128;
    partition row p of tile t owns exactly one destination node. The output
    is un-permuted on the host.
  - Node table tbl[n] = [xl(n) bf16(128) | s_a(4) | s_b(4) | pad..256] with
    xl = x @ W_lin^T, s_a = xl.att_src (dst term), s_b = xl.att_dst (src
    term), built on device (phase B) from a host-transposed bf16 x.
    Rows are 512 B so one dma_gather descriptor moves a full row.
  - Edge phase, per tile: each dst node's edges occupy its partition row,
    padded to the tile max degree D. Source rows are fetched with TWO
    batched dma_gather calls (int16 idx limit -> lo nodes [0,25088) /
    hi nodes [25088,50048) tables), writing adjacent column ranges of one
    SBUF tile. alpha = lrelu(ae + s_b[src] + s_a[own]); pad slots carry a
    host-solved edge_attr vector v with v.C = -100 so e = exp(lrelu(...))
    underflows to ~2e-9 (no masks). msg = e * xl_src; numerator+denominator
    reduced over the edge axis on DVE; epilogue: divide, +bias, +residual,
    LayerNorm, ELU, store.
  - No per-edge matmuls, no one-hots, no collectives; params host-folded
    (rhsB = [W_lin^T | W_lin^T A_src | W_lin^T A_dst], C = W_edge^T A_e).
"""

import math

import numpy as np
import ml_dtypes

import concourse.bass as bass
import concourse.bacc as bacc
import concourse.mybir as mybir
from concourse.tile import TileContext
from concourse import library_config
from concourse.bass_utils import run_bass_kernel_spmd

F32 = mybir.dt.float32
BF16 = mybir.dt.bfloat16
I16 = mybir.dt.int16
AF = mybir.ActivationFunctionType
OP = mybir.AluOpType
AX = mybir.AxisListType

H, C = 4, 32
HC = H * C          # 128
IN_CH = 128
ED = 16
NEG_SLOPE = 0.2
LN_EPS = 1e-5
P = 128
PAD_ALPHA = -100.0  # pad-slot ae term; lrelu -> -20, exp -> 2e-9

FULL_CFG = dict(n_nodes=50000, n_cores=8, n_edges=1600000)


def derive_cfg(cfg):
    n, cores = cfg["n_nodes"], cfg["n_cores"]
    npc = n // cores
    assert npc * cores == n
    tiles = math.ceil(npc / P)
    npad = tiles * P
    nt_tbl = math.ceil(n / P)       # global table tiles (391)
    n_tbl = nt_tbl * P              # 50048
    # Overlapping int16-addressable tables:
    #   tbl_lo = rows [0, 32768)          (tiles 0..255)
    #   tbl_hi = rows [hi_base, n_tbl)    (tiles hi_t0..390, 32768 rows)
    lo_sz = 256 * P                 # 32768
    hi_base = n_tbl - 32768         # 17280, tile-aligned (135*128)
    hi_t0 = hi_base // P            # 135
    # edges with src in [hi_base, lo_sz) may go to either table
    return dict(cfg, npc=npc, tiles=tiles, npad=npad, nt_tbl=nt_tbl,
                n_tbl=n_tbl, lo_sz=lo_sz, hi_base=hi_base, hi_t0=hi_t0)


# --------------------------------------------------------------------------
# host-side routing / layout (index bookkeeping + dtype casts only)
# --------------------------------------------------------------------------

def host_prep(x, edge_index, edge_attr, W_lin, W_edge, att_src, att_dst,
              att_edge, bias, ln_gamma, ln_beta, cfg):
    dc = derive_cfg(cfg)
    cores, npc, tiles, npad = dc["n_cores"], dc["npc"], dc["tiles"], dc["npad"]
    n, n_tbl = dc["n_nodes"], dc["n_tbl"]
    lo_sz, hi_base = dc["lo_sz"], dc["hi_base"]

    x = np.asarray(x, np.float32)
    src = np.asarray(edge_index[0], np.int64)
    dst = np.asarray(edge_index[1], np.int64)
    ea = np.asarray(edge_attr, np.float32)

    W_lin = np.asarray(W_lin, np.float32)
    W_edge = np.asarray(W_edge, np.float32)
    a_src = np.asarray(att_src, np.float32).reshape(H, C)
    a_dst = np.asarray(att_dst, np.float32).reshape(H, C)
    a_edge = np.asarray(att_edge, np.float32).reshape(H, C)

    # ---- folded params -------------------------------------------------
    # rhsB[:, 0:128] = W_lin^T ; [:,128:132] = W_lin^T A_src ; [:,132:136]
    # = W_lin^T A_dst   (A_* block-diagonal [HC, H])
    A_srcM = np.zeros((HC, H), np.float32)
    A_dstM = np.zeros((HC, H), np.float32)
    A_edgeM = np.zeros((HC, H), np.float32)
    for h in range(H):
        A_srcM[h * C:(h + 1) * C, h] = a_src[h]
        A_dstM[h * C:(h + 1) * C, h] = a_dst[h]
        A_edgeM[h * C:(h + 1) * C, h] = a_edge[h]
    rhsB = np.concatenate(
        [W_lin.T, W_lin.T @ A_srcM, W_lin.T @ A_dstM], axis=1)  # [128,136]
    Cmat = W_edge.T @ A_edgeM                                    # [16, 4]
    # pad-slot edge_attr vector: v @ Cmat = PAD_ALPHA * ones(4)
    v_pad = np.linalg.lstsq(Cmat.T, PAD_ALPHA * np.ones(H), rcond=None)[0]

    # ---- per-core routing ---------------------------------------------
    core = dst // npc
    lcl = (dst - core * npc).astype(np.int64)
    gnode = core * npc + lcl

    # lo/hi table assignment: srcs in the overlap [hi_base, lo_sz) are
    # flexible; assign them per dst node to balance lo vs hi degree.
    fixed_lo = src < hi_base
    fixed_hi = src >= lo_sz
    flex = (~fixed_lo) & (~fixed_hi)
    nl_f = np.bincount(gnode[fixed_lo], minlength=cores * npc)
    nh_f = np.bincount(gnode[fixed_hi], minlength=cores * npc)
    nfl = np.bincount(gnode[flex], minlength=cores * npc)
    x_lo = np.clip((nh_f + nfl - nl_f + 1) // 2, 0, nfl)  # flex edges -> lo

    is_lo = fixed_lo.copy()
    fidx = np.nonzero(flex)[0]
    o3 = np.argsort(gnode[fidx], kind="stable")
    ks = gnode[fidx][o3]
    counts3 = np.bincount(ks, minlength=cores * npc)
    gstart3 = np.zeros(cores * npc, np.int64)
    np.cumsum(counts3[:-1], out=gstart3[1:])
    frank = np.arange(len(fidx)) - gstart3[ks]
    is_lo[fidx[o3]] = frank < x_lo[ks]

    # per (core, local node) degrees
    deg = np.zeros((cores, npc), np.int64)
    np.add.at(deg, (core, lcl), 1)
    deg_lo = np.zeros((cores, npc), np.int64)
    np.add.at(deg_lo, (core, lcl), is_lo)
    deg_hi = deg - deg_lo

    # degree-sorted permutation per core
    order = np.argsort(-deg, axis=1, kind="stable")      # [cores, npc]
    pos = np.empty_like(order)
    for c in range(cores):
        pos[c, order[c]] = np.arange(npc)
    # per-tile max degrees, unified across cores
    def tile_max(d):
        ds = np.take_along_axis(d, order, axis=1)        # sorted by pos
        ds = np.pad(ds, ((0, 0), (0, npad - npc)))
        return ds.reshape(cores, tiles, P).max(axis=2).max(axis=0)
    D_lo = np.maximum(tile_max(deg_lo), 1).astype(np.int64)   # [tiles]
    D_hi = np.maximum(tile_max(deg_hi), 1).astype(np.int64)
    D_t = D_lo + D_hi
    colbase = np.zeros(tiles + 1, np.int64)
    np.cumsum(D_t, out=colbase[1:])
    sum_d = int(colbase[-1])
    # idx column bases (per tile: 8*D_lo lo-cols then 8*D_hi hi-cols)
    iw_t = 8 * D_t
    ibase = np.zeros(tiles + 1, np.int64)
    np.cumsum(iw_t, out=ibase[1:])
    iw = int(ibase[-1])

    # per-edge placement
    p_e = pos[core, lcl]                       # permuted position in core
    t_e = p_e // P
    r_e = p_e % P
    # rank within (core, node, side)
    key = (core * npc + p_e) * 2 + (~is_lo)
    o2 = np.argsort(key, kind="stable")
    key_s = key[o2]
    counts = np.bincount(key_s, minlength=2 * cores * npc)
    gstart = np.zeros(2 * cores * npc, np.int64)
    np.cumsum(counts[:-1], out=gstart[1:])
    k_e = np.empty(len(src), np.int64)
    k_e[o2] = np.arange(len(src)) - gstart[key_s]

    # ---- build per-core arrays ----------------------------------------
    x_bf = x.astype(ml_dtypes.bfloat16)
    xT_glob = np.zeros((IN_CH, n_tbl), ml_dtypes.bfloat16)
    xT_glob[:, :n] = x_bf.T

    ea_bf = ea.astype(ml_dtypes.bfloat16)
    v_pad_bf = v_pad.astype(ml_dtypes.bfloat16)

    in_maps = []
    perms = []
    for c in range(cores):
        sel = core == c
        sc_src = src[sel]
        sc_t, sc_r, sc_k = t_e[sel], r_e[sel], k_e[sel]
        sc_lo = is_lo[sel]
        sc_ea = ea_bf[sel]

        # ea slots: [128, sum_d, ED], pad slots = v_pad
        ea_all = np.broadcast_to(v_pad_bf, (P, sum_d, ED)).copy()
        col = np.where(sc_lo, sc_k, D_lo[sc_t] + sc_k) + colbase[sc_t]
        ea_all[sc_r, col, :] = sc_ea

        # gather idx arrays (int16, 16-wrapped, replicated to 128 rows)
        idx_all = np.zeros((P, iw), np.int16)
        flat = np.zeros(8 * 2 * sum_d * 8, np.int16)  # overalloc scratch
        for t in range(tiles):
            m = sc_t == t
            for side, (dd, base_off) in enumerate(
                    [(int(D_lo[t]), 0), (int(D_hi[t]), 8 * int(D_lo[t]))]):
                ms = m & (sc_lo if side == 0 else ~sc_lo)
                slot = sc_k[ms] * P + sc_r[ms]
                vals = sc_src[ms] if side == 0 else sc_src[ms] - hi_base
                arr = np.zeros(P * dd, np.int16)
                arr[slot] = vals.astype(np.int16)
                wrapped = arr.reshape(-1, 16).T      # [16, 8*dd]
                cols = np.tile(wrapped, (8, 1))      # [128, 8*dd]
                c0 = ibase[t] + base_off
                idx_all[:, c0:c0 + 8 * dd] = cols

        # residual + own-node features (permuted order)
        g_ids = c * npc + order[c]                   # global node per pos
        xres = np.zeros((npad, IN_CH), np.float32)
        xres[:npc] = x[g_ids]
        xTp = np.zeros((IN_CH, npad), ml_dtypes.bfloat16)
        xTp[:, :npc] = x_bf[g_ids].T

        in_maps.append(dict(
            xT_glob=xT_glob,
            xTp=xTp,
            xres=xres,
            idx_all=idx_all,
            ea_all=np.ascontiguousarray(
                ea_all.reshape(P, sum_d * ED)),
            rhsB=rhsB.astype(ml_dtypes.bfloat16),
            ct_row=np.ascontiguousarray(
                Cmat.T.reshape(1, H * ED)).astype(ml_dtypes.bfloat16),
            bias=np.asarray(bias, np.float32).reshape(1, HC),
            ln_gamma=np.asarray(ln_gamma, np.float32).reshape(1, HC),
            ln_beta=np.asarray(ln_beta, np.float32).reshape(1, HC),
        ))
        perms.append(order[c])

    meta = dict(D_lo=[int(v) for v in D_lo], D_hi=[int(v) for v in D_hi],
                colbase=[int(v) for v in colbase],
                ibase=[int(v) for v in ibase],
                sum_d=sum_d, iw=iw, perms=perms)
    return in_maps, meta, dc


# --------------------------------------------------------------------------
# device program
# --------------------------------------------------------------------------

def build_program(meta, cfg, num_devices=None):
    dc = derive_cfg(cfg)
    tiles, npad, nt_tbl = dc["tiles"], dc["npad"], dc["nt_tbl"]
    n_tbl, lo_sz, hi_base, hi_t0 = (dc["n_tbl"], dc["lo_sz"], dc["hi_base"],
                                    dc["hi_t0"])
    D_lo, D_hi = meta["D_lo"], meta["D_hi"]
    colbase, ibase = meta["colbase"], meta["ibase"]
    sum_d, iw = meta["sum_d"], meta["iw"]
    D_max = max(dl + dh for dl, dh in zip(D_lo, D_hi))
    TW = 136  # used row width: xl(128) | s_a(4) | s_b(4)
    RW = 256  # stored row width (512B)

    nc = bacc.Bacc("TRN2", target_bir_lowering=False, debug=False,
                   num_devices=num_devices or dc["n_cores"])

    dp = nc.declare_dram_parameter
    xTg_d = dp("xT_glob", [IN_CH, n_tbl], BF16, isOutput=False)
    xTp_d = dp("xTp", [IN_CH, npad], BF16, isOutput=False)
    xres_d = dp("xres", [npad, IN_CH], F32, isOutput=False)
    idx_d = dp("idx_all", [P, iw], I16, isOutput=False)
    ea_d = dp("ea_all", [P, sum_d * ED], BF16, isOutput=False)
    rhsB_d = dp("rhsB", [IN_CH, TW], BF16, isOutput=False)
    ct_d = dp("ct_row", [1, H * ED], BF16, isOutput=False)
    bias_d = dp("bias", [1, HC], F32, isOutput=False)
    gamma_d = dp("ln_gamma", [1, HC], F32, isOutput=False)
    beta_d = dp("ln_beta", [1, HC], F32, isOutput=False)
    out_d = dp("out", [npad, HC], F32, isOutput=True)

    tbl_lo = nc.dram_tensor("tbl_lo", [lo_sz, RW], BF16)
    tbl_hi = nc.dram_tensor("tbl_hi", [n_tbl - hi_base, RW], BF16)

    with TileContext(nc) as tc:
        with (
            tc.tile_pool(name="const", bufs=1) as cpool,
            tc.tile_pool(name="bwork", bufs=3) as bpool,
            tc.tile_pool(name="gath", bufs=3) as gpool,
            tc.tile_pool(name="work", bufs=2) as wpool,
            tc.tile_pool(name="psB", bufs=3, space="PSUM") as psB,
            tc.tile_pool(name="psC", bufs=2, space="PSUM") as psC,
        ):
            nc.gpsimd.load_library(library_config.mlp)

            # ---------------- constants --------------------------------
            rhsB = cpool.tile([IN_CH, TW], BF16, tag="rhsB")
            nc.sync.dma_start(out=rhsB[:], in_=rhsB_d[:])
            ctb = cpool.tile([P, H * ED], BF16, tag="ctb")
            nc.sync.dma_start(out=ctb[:], in_=ct_d[:].to_broadcast([P, H * ED]))
            bias_b = cpool.tile([P, HC], F32, tag="bias_b")
            nc.sync.dma_start(out=bias_b[:], in_=bias_d[:].to_broadcast([P, HC]))
            gamma_b = cpool.tile([P, HC], F32, tag="gamma_b")
            nc.sync.dma_start(out=gamma_b[:],
                              in_=gamma_d[:].to_broadcast([P, HC]))
            beta_b = cpool.tile([P, HC], F32, tag="beta_b")
            nc.sync.dma_start(out=beta_b[:], in_=beta_d[:].to_broadcast([P, HC]))
            eps_t = cpool.tile([P, 1], F32, tag="eps_t")
            nc.vector.memset(eps_t[:], LN_EPS)
            tiny_t = cpool.tile([P, 1], F32, tag="tiny_t")
            nc.vector.memset(tiny_t[:], 1e-16)

            # ---------------- phase B: node table ----------------------
            BB = 8  # tiles per batch
            for t0 in range(0, nt_tbl, BB):
                tb = min(BB, nt_tbl - t0)
                xT = bpool.tile([IN_CH, BB * P], BF16, tag="xT")
                nc.sync.dma_start(out=xT[:, 0:tb * P],
                                  in_=xTg_d[:, t0 * P:(t0 + tb) * P])
                row = bpool.tile([P, BB, RW], BF16, tag="row")
                nc.vector.memset(row[:, 0:tb, TW:RW], 0.0)
                for j in range(tb):
                    row_ps = psB.tile([P, TW], F32, tag="ps")
                    nc.tensor.matmul(out=row_ps[:],
                                     lhsT=xT[:, j * P:(j + 1) * P],
                                     rhs=rhsB[:], start=True, stop=True)
                    nc.scalar.copy(out=row[:, j, 0:TW], in_=row_ps[:])
                # contiguous table-row writes, split at the lo/hi boundary
                jl = min(tb, max(0, (lo_sz // P) - t0))
                if jl > 0:
                    nc.scalar.dma_start(
                        out=tbl_lo[t0 * P:(t0 + jl) * P, :]
                        .rearrange("(t p) c -> p t c", p=P),
                        in_=row[:, 0:jl, :])
                js = max(0, hi_t0 - t0)
                if js < tb:
                    a = t0 + js - hi_t0
                    nc.scalar.dma_start(
                        out=tbl_hi[a * P:(a + tb - js) * P, :]
                        .rearrange("(t p) c -> p t c", p=P),
                        in_=row[:, js:tb, :])

            # ---------------- phase C: edges ---------------------------
            for t in range(tiles):
                dl, dh = D_lo[t], D_hi[t]
                d = dl + dh
                cb, ib = colbase[t], ibase[t]

                idx_sb = wpool.tile([P, 8 * D_max + 16], I16, tag="idx")
                nc.sync.dma_start(out=idx_sb[:, 0:8 * d],
                                  in_=idx_d[:, ib:ib + 8 * d])
                ea_sb = wpool.tile([P, D_max, ED], BF16, tag="ea")
                nc.sync.dma_start(
                    out=ea_sb[:, 0:d, :].rearrange("p d e -> p (d e)"),
                    in_=ea_d[:, cb * ED:(cb + d) * ED])
                xr = wpool.tile([P, IN_CH], F32, tag="xr")
                nc.sync.dma_start(out=xr[:], in_=xres_d[t * P:(t + 1) * P, :])

                # gathers chunked to <=1024 idx (HW SWDGE packet limit)
                g = gpool.tile([P, D_max, RW], BF16, tag="g")
                GC = 8  # columns (128 idx each) per gather
                for c0 in range(0, dl, GC):
                    k = min(GC, dl - c0)
                    nc.gpsimd.dma_gather(
                        g[:, c0:c0 + k, :], tbl_lo[:],
                        idx_sb[:, 8 * c0:8 * (c0 + k)],
                        P * k, P * k, RW)
                for c0 in range(0, dh, GC):
                    k = min(GC, dh - c0)
                    nc.gpsimd.dma_gather(
                        g[:, dl + c0:dl + c0 + k, :], tbl_hi[:],
                        idx_sb[:, 8 * (dl + c0):8 * (dl + c0 + k)],
                        P * k, P * k, RW)

                # own-node s_a via small PE matmul: [128n, 8]
                sown_ps = psC.tile([P, 2 * H], F32, tag="sown")
                xtp_sb = wpool.tile([IN_CH, P], BF16, tag="xtp")
                nc.sync.dma_start(out=xtp_sb[:],
                                  in_=xTp_d[:, t * P:(t + 1) * P])
                nc.tensor.matmul(out=sown_ps[:], lhsT=xtp_sb[:],
                                 rhs=rhsB[:, HC:HC + 2 * H],
                                 start=True, stop=True)
                sown = wpool.tile([P, 2 * H], F32, tag="sownb")
                nc.scalar.copy(out=sown[:], in_=sown_ps[:])

                # alpha[p, d, h] = ea . C  (per head) + s_b(src) + s_a(own)
                alpha = wpool.tile([P, D_max, H], F32, tag="alpha")
                prod = wpool.tile([P, D_max, ED], BF16, tag="prod")
                eav = ea_sb[:, 0:d, :]
                for h in range(H):
                    ctb_h = (ctb[:, h * ED:(h + 1) * ED]
                             .unsqueeze(1).to_broadcast([P, d, ED]))
                    nc.vector.tensor_tensor(
                        out=prod[:, 0:d, :], in0=eav, in1=ctb_h, op=OP.mult)
                    nc.vector.reduce_sum(
                        out=alpha[:, 0:d, h:h + 1], in_=prod[:, 0:d, :],
                        axis=AX.X)
                nc.vector.tensor_tensor(
                    out=alpha[:, 0:d, :], in0=alpha[:, 0:d, :],
                    in1=g[:, 0:d, HC + H:HC + 2 * H], op=OP.add)
                nc.vector.tensor_tensor(
                    out=alpha[:, 0:d, :], in0=alpha[:, 0:d, :],
                    in1=sown[:, 0:H].unsqueeze(1).to_broadcast([P, d, H]),
                    op=OP.add)
                # leaky_relu, then exp (f32)
                nc.vector.scalar_tensor_tensor(
                    out=alpha[:, 0:d, :], in0=alpha[:, 0:d, :],
                    scalar=NEG_SLOPE, in1=alpha[:, 0:d, :],
                    op0=OP.mult, op1=OP.max)
                ex = wpool.tile([P, D_max, H], F32, tag="ex")
                nc.scalar.activation(out=ex[:, 0:d, :], in_=alpha[:, 0:d, :],
                                     func=AF.Exp)

                # msg[p, d, 0:128] = e * xl_src ; [:,:,128:132] = e
                msg = wpool.tile([P, D_max, HC + H], BF16, tag="msg")
                for h in range(H):
                    nc.vector.tensor_tensor(
                        out=msg[:, 0:d, h * C:(h + 1) * C],
                        in0=g[:, 0:d, h * C:(h + 1) * C],
                        in1=ex[:, 0:d, h:h + 1].to_broadcast([P, d, C]),
                        op=OP.mult)
                nc.vector.tensor_copy(out=msg[:, 0:d, HC:HC + H],
                                      in_=ex[:, 0:d, :])

                # reduce over edge axis: binary tree, f32 accumulator
                acc = wpool.tile([P, (D_max + 1) // 2, HC + H], F32,
                                 tag="acc")
                hh = d // 2
                nc.vector.tensor_tensor(
                    out=acc[:, 0:hh, :], in0=msg[:, 0:hh, :],
                    in1=msg[:, hh:2 * hh, :], op=OP.add)
                cur = hh
                if d % 2:
                    nc.vector.tensor_copy(out=acc[:, hh:hh + 1, :],
                                          in_=msg[:, 2 * hh:2 * hh + 1, :])
                    cur = hh + 1
                while cur > 1:
                    hh = cur // 2
                    nc.vector.tensor_tensor(
                        out=acc[:, 0:hh, :], in0=acc[:, 0:hh, :],
                        in1=acc[:, hh:2 * hh, :], op=OP.add)
                    if cur % 2:
                        nc.vector.tensor_tensor(
                            out=acc[:, 0:1, :], in0=acc[:, 0:1, :],
                            in1=acc[:, 2 * hh:2 * hh + 1, :], op=OP.add)
                    cur = hh

                red = acc[:, 0, :]          # [P, 132] f32 view

                # ---------------- epilogue -----------------------------
                den = wpool.tile([P, H], F32, tag="den")
                nc.scalar.activation(out=den[:], in_=red[:, HC:HC + H],
                                     func=AF.Identity, bias=tiny_t[:, 0:1])
                rden = wpool.tile([P, H], F32, tag="rden")
                nc.vector.reciprocal(out=rden[:], in_=den[:])
                o = wpool.tile([P, HC], F32, tag="o")
                for h in range(H):
                    nc.scalar.activation(out=o[:, h * C:(h + 1) * C],
                                         in_=red[:, h * C:(h + 1) * C],
                                         func=AF.Copy,
                                         scale=rden[:, h:h + 1])
                nc.vector.tensor_add(out=o[:], in0=o[:], in1=bias_b[:])
                nc.vector.tensor_add(out=o[:], in0=o[:], in1=xr[:])

                mu = wpool.tile([P, 1], F32, tag="mu")
                nc.vector.reduce_sum(out=mu[:], in_=o[:], axis=AX.X)
                nc.scalar.mul(out=mu[:], in_=mu[:], mul=1.0 / HC)
                ctr = wpool.tile([P, HC], F32, tag="ctr")
                nc.vector.tensor_scalar_sub(out=ctr[:], in0=o[:],
                                            scalar1=mu[:, 0:1])
                sq = wpool.tile([P, HC], F32, tag="sq")
                var = wpool.tile([P, 1], F32, tag="var")
                nc.vector.tensor_mul(out=sq[:], in0=ctr[:], in1=ctr[:])
                nc.vector.reduce_sum(out=var[:], in_=sq[:], axis=AX.X)
                nc.scalar.mul(out=var[:], in_=var[:], mul=1.0 / HC)
                std = wpool.tile([P, 1], F32, tag="std")
                nc.scalar.activation(out=std[:], in_=var[:], func=AF.Sqrt,
                                     bias=eps_t[:, 0:1])
                rstd = wpool.tile([P, 1], F32, tag="rstd")
                nc.vector.reciprocal(out=rstd[:], in_=std[:])
                nrm = wpool.tile([P, HC], F32, tag="nrm")
                nc.scalar.activation(out=nrm[:], in_=ctr[:], func=AF.Copy,
                                     scale=rstd[:, 0:1])
                nc.vector.tensor_mul(out=nrm[:], in0=nrm[:], in1=gamma_b[:])
                nc.vector.tensor_add(out=nrm[:], in0=nrm[:], in1=beta_b[:])

                exq = wpool.tile([P, HC], F32, tag="exq")
                nc.scalar.activation(out=exq[:], in_=nrm[:], func=AF.Exp)
                nc.vector.tensor_scalar(out=exq[:], in0=exq[:], scalar1=-1.0,
                                        scalar2=0.0, op0=OP.add, op1=OP.min)
                rl = wpool.tile([P, HC], F32, tag="rl")
                nc.scalar.activation(out=rl[:], in_=nrm[:], func=AF.Relu)
                nc.vector.tensor_add(out=rl[:], in0=rl[:], in1=exq[:])
                nc.sync.dma_start(out=out_d[t * P:(t + 1) * P, :], in_=rl[:])

    nc.compile()
    return nc


# --------------------------------------------------------------------------
# entry point
# --------------------------------------------------------------------------

def kernel(**inputs) -> np.ndarray:
    cfg = FULL_CFG
    in_maps, meta, dc = host_prep(cfg=cfg, **inputs)
    nc = build_program(meta, cfg)
    cores, npc = cfg["n_cores"], dc["npc"]
    res = run_bass_kernel_spmd(nc, in_maps, list(range(cores)))
    out = np.empty((cfg["n_nodes"], HC), np.float32)
    for c in range(cores):
        part = np.asarray(res.results[c]["out"][:npc], np.float32)
        out[c * npc + meta["perms"][c]] = part
    return out
